# revision 6
# baseline (speedup 1.0000x reference)
"""Trainium2 Bass kernel for nn_Actor (MTRNN actor: 4-step LSTM stack + Bernoulli head).

Data-parallel over 8 NeuronCores: batch 4096 -> 512 rows/core, weights replicated.
Everything on-chip lives in [feature, batch] (transposed) layout; all transposes and
weight tiling happen on the host so the NEFF contains only matmuls + elementwise.

Precision: LSTM1 (x-matmul + recurrent) in fp8 e4m3 operands with DoubleRow perf
mode (2 fp8 MACs per PE cell per cycle) and fp32 PSUM accumulation; weights are
host-scaled by 64 to stay in e4m3 normal range (undone by the activation's scale).
LSTM2-4 stay bf16 and the output head fp32 — fp8 there costs action-bit flips
(validated host-side: this split flips 2 of 2.1M action bits vs fp32, well inside
the rel_err 2e-2 gate).
"""
import sys
from contextlib import ExitStack

import numpy as np

sys.path.insert(0, "/opt/trn_rl_repo")

import ml_dtypes

import concourse.bass as bass
import concourse.tile as tile
from concourse import bacc, mybir
from concourse.vector_clock import ScopedClock

BF16 = ml_dtypes.bfloat16
E4M3 = ml_dtypes.float8_e4m3

H = 512
T = 4
IN_DIM = 3072
ACT_DIM = 512
B = 4096
N_CORES = 8
BL = B // N_CORES          # 512 batch rows per core
KX = IN_DIM // 128         # 24 input-feature chunks
KP = KX // 2               # 12 DoubleRow k-pairs
NM = 16                    # gate chunks (4H/128)
NR = 4                     # H row chunks
WSCALE = 64.0              # host-side weight scale into e4m3 normal range

F32 = mybir.dt.float32
BF = mybir.dt.bfloat16
FP8 = mybir.dt.float8e4
Act = mybir.ActivationFunctionType
Alu = mybir.AluOpType
DR = mybir.MatmulPerfMode.DoubleRow


# ---------------------------------------------------------------------------
# TileContext drain patch: this walrus caps sync-waits per instruction, while
# the stock Tile exit puts one wait per live semaphore on a single Drain.
# Redistribute: one nop per wait, then a wait-free drain.
# ---------------------------------------------------------------------------
def _split_drain_and_barrier(self, tick_clock, wait_clock):
    nc = self.nc
    collector = nc.sync.nop(nofuse=True)
    wait_clock.add_sem_waits(collector.ins, ScopedClock({None: tick_clock.global_clock}))
    si = collector.ins.sync_info
    waits = list(si.on_wait) if si is not None else []
    if len(waits) > 1:
        collector.ins.sync_info = None
        id2sem = {h.num: h for h in self.sems.allocated().values()}
        for w in waits:
            nc.sync.nop(nofuse=True).wait_op(id2sem[w.id], w.wait_value, "sem-ge")
    nc.sync.drain()
    nc.all_engine_barrier()
    assert self.sems is not None
    popped = nc._tile_sem_poison_stack.pop()
    assert popped is self._sem_poison
    nc.clear_and_free_semaphores(list(self.sems.allocated().values()))
    nc.all_engine_barrier()


tile.TileContext._drain_and_barrier = _split_drain_and_barrier


def _chunk(role: str, r: int) -> int:
    """Gate chunk index for role in torch LSTM order [i, f, g, o]."""
    return {"i": 0, "f": 1, "g": 2, "o": 3}[role] * NR + r


def build_kernel() -> bass.Bass:
    nc = bacc.Bacc()

    KH = KP // 2            # 6 k-pairs per x half-tile

    x_ext = nc.declare_dram_parameter("x", [T, 2, 128, 2 * KH, BL], FP8, isOutput=False)
    w1_ext = nc.declare_dram_parameter("w1", [NM, 128, KX, 128], FP8, isOutput=False)
    wh1_ext = nc.declare_dram_parameter("wh1", [128, NM * 4, 128], FP8, isOutput=False)
    w2_ext = nc.declare_dram_parameter("w2", [128, NM * 512], BF, isOutput=False)
    w3_ext = nc.declare_dram_parameter("w3", [128, NM * 512], BF, isOutput=False)
    w4_ext = nc.declare_dram_parameter("w4", [128, NM * 512], BF, isOutput=False)
    wo_ext = nc.declare_dram_parameter("wo", [128, NR * 512], F32, isOutput=False)
    bias_ext = nc.declare_dram_parameter("bias", [128, 68], F32, isOutput=False)
    u_ext = nc.declare_dram_parameter("u", [128, NR * BL], F32, isOutput=False)
    out_ext = nc.declare_dram_parameter("out", [NR, 128, BL], F32, isOutput=True)

    with ExitStack() as ctx:
        tc = ctx.enter_context(tile.TileContext(nc))
        pers = ctx.enter_context(tc.tile_pool(name="pers", bufs=1))
        gate = ctx.enter_context(tc.tile_pool(name="gate", bufs=12))
        ps = ctx.enter_context(tc.tile_pool(name="ps", bufs=8, space="PSUM"))

        def load_x_step(t):
            tiles = []
            for s in range(2):
                xt = pers.tile([128, 2 * KH, BL], FP8, name=f"x_t{t}_{s}",
                               tag=f"xh{s}", bufs=2)
                # t0 halves go down the two HWDGE rings (sync + scalar) whose
                # issuing engines are idle at start; later steps stay on the
                # gpsimd SWDGE ring, which starts slowest and must never gate
                # a t0-critical semaphore
                eng = (nc.sync, nc.scalar)[s] if t == 0 else nc.gpsimd
                eng.dma_start(xt[:], x_ext[t][s])
                tiles.append(xt)
            return tiles

        x_step = load_x_step(0)
        bias_sb = pers.tile([128, 68], F32, name="bias", tag="bias")
        nc.gpsimd.dma_start(bias_sb[:], bias_ext[:])
        w1_tiles = {}           # gate-chunk m -> persistent fp8 weight tile

        def load_w1(m, eng):
            w1t = pers.tile([128, KX, 128], FP8, name=f"w1_{m}", tag=f"w1_{m}")
            eng.dma_start(w1t[:], w1_ext[m])
            w1_tiles[m] = w1t

        # all 16 w1 tiles up front, balanced across the three rings (~150GB/s
        # each) in t0 usage order; the 4 f tiles (first needed at t1) trail on
        # the scalar ring behind x0's second half
        for m in (0, 8, 12, 1, 9, 13):
            load_w1(m, nc.sync)
        for m in (2, 10, 14):
            load_w1(m, nc.scalar)
        for m in (3, 11, 15):
            load_w1(m, nc.gpsimd)
        for m in (4, 5, 6, 7):
            load_w1(m, nc.scalar)
        warm = pers.tile([128, 1], F32, name="warm", tag="warm")
        nc.scalar.activation(warm[:], bias_sb[:, 0:1], Act.Sigmoid)
        nc.scalar.activation(warm[:], bias_sb[:, 0:1], Act.Tanh)
        dmy_w = pers.tile([128, 128], BF, name="dmy_w", tag="dmy_w")
        nc.vector.memset(dmy_w[:], 0.0)
        dmy_x = pers.tile([128, BL], BF, name="dmy_x", tag="dmy_x")
        nc.vector.memset(dmy_x[:], 0.0)
        dmy_p = ps.tile([128, BL], F32, name="dmy_p", tag="psum")
        for _ in range(10):
            nc.tensor.matmul(dmy_p[:], dmy_w[:], dmy_x[:], start=True, stop=True)
        nc.scalar.activation(warm[:], dmy_p[:, 0:1], Act.Relu)
        wh1_sb = pers.tile([128, NM * 4, 128], FP8, name="wh1", tag="wh1")
        nc.gpsimd.dma_start(wh1_sb[:], wh1_ext[:])

        # persistent state
        c1 = [pers.tile([128, BL], F32, name=f"c1_{r}", tag=f"c1_{r}") for r in range(NR)]
        h1 = [pers.tile([128, BL], BF, name=f"h1_{r}", tag=f"h1_{r}") for r in range(NR)]
        h1p = pers.tile([128, NR, BL], FP8, name="h1p", tag="h1p")
        h2 = [pers.tile([128, BL], BF, name=f"h2_{r}", tag=f"h2_{r}") for r in range(NR)]
        h3 = [pers.tile([128, BL], BF, name=f"h3_{r}", tag=f"h3_{r}") for r in range(NR)]
        h4 = []
        wl_tiles = {}
        u_sb = None
        INV = 1.0 / WSCALE

        def bias_ap(col):
            return bias_sb[:, col:col + 1]

        def emit_lblock(idx, src_h):
            wl = wl_tiles[("w2", "w3", "w4")[idx]]
            if idx == 2 and not h4:
                h4.extend(
                    pers.tile([128, BL], mybir.dt.float32r, name=f"h4_{r}", tag=f"h4_{r}")
                    for r in range(NR)
                )
            dst = (h2, h3, h4)[idx]
            bias_off = 16 * (idx + 1)
            for half in ((0, 1), (2, 3)):
                psums = {}
                for r in half:
                    for role in ("i", "g", "o"):     # f-gate unused (c_prev=0)
                        m = _chunk(role, r)
                        p = ps.tile([128, BL], F32, name="psum", tag="psum")
                        psums[(role, r)] = p
                        for k in range(NR):
                            nc.tensor.matmul(
                                p[:],
                                wl[:, m * 512 + k * 128:m * 512 + (k + 1) * 128],
                                src_h[k][:],
                                start=(k == 0),
                                stop=(k == NR - 1),
                            )
                for r in half:
                    si = gate.tile([128, BL], F32, name="si", tag="gt")
                    nc.scalar.activation(si[:], psums[("i", r)][:], Act.Sigmoid,
                                         bias=bias_ap(bias_off + _chunk("i", r)))
                    tg = gate.tile([128, BL], F32, name="tg", tag="gt")
                    nc.scalar.activation(tg[:], psums[("g", r)][:], Act.Tanh,
                                         bias=bias_ap(bias_off + _chunk("g", r)))
                    so = gate.tile([128, BL], F32, name="so", tag="gt")
                    nc.scalar.activation(so[:], psums[("o", r)][:], Act.Sigmoid,
                                         bias=bias_ap(bias_off + _chunk("o", r)))
                    cn = gate.tile([128, BL], F32, name="ig", tag="gt")
                    nc.vector.tensor_tensor(cn[:], si[:], tg[:], Alu.mult)
                    tc_ = gate.tile([128, BL], F32, name="tc", tag="gt")
                    nc.scalar.activation(tc_[:], cn[:], Act.Tanh)
                    nc.vector.scalar_tensor_tensor(dst[r][:], tc_[:], 0.0, so[:],
                                                   Alu.max, Alu.mult)

        # --- LSTM1: T fused steps (fp8 DoubleRow matmuls) ---------------------
        for t in range(T):
            roles = ("i", "g", "o") if t == 0 else ("i", "f", "g", "o")
            x_cur = x_step
            if t + 1 < T:
                x_step = load_x_step(t + 1)
            # prefetch the L-block weight for this step's tail (w2/w3/w4),
            # plus head tensors at the last step
            if t >= 1:
                name = ("w2", "w3", "w4")[t - 1]
                wlt = pers.tile([128, NM * 512], BF, name=name, tag="wl", bufs=2)
                nc.gpsimd.dma_start(wlt[:], (w2_ext, w3_ext, w4_ext)[t - 1][:])
                wl_tiles[name] = wlt
            if t > 0:
                for r in range(NR):
                    nc.vector.tensor_copy(h1p[:, r, :], h1[r][:])
            if t == T - 1:
                wo_sb = pers.tile([128, NR * 512], mybir.dt.float32r, name="wo", tag="wo")
                nc.gpsimd.dma_start(wo_sb[:], wo_ext[:])
                u_sb = pers.tile([128, NR * BL], F32, name="u", tag="u")
                nc.gpsimd.dma_start(u_sb[:], u_ext[:])

            for half in ((0, 1), (2, 3)):
                psums = {}

                def xs(kp):
                    # rhs pair kp (0..KP-1): [128, 2, BL] slice of an x half-tile
                    s, kk = kp // KH, kp % KH
                    return x_cur[s][:, 2 * kk:2 * kk + 2, :]

                def emit_group(r, role, kps):
                    m = _chunk(role, r)
                    if (role, r) not in psums:
                        psums[(role, r)] = ps.tile([128, BL], F32,
                                                   name="psum", tag="psum")
                    p, w1t = psums[(role, r)], w1_tiles[m]
                    for kp in kps:
                        nc.tensor.matmul(
                            p[:],
                            w1t[:, 2 * kp:2 * kp + 2, :],
                            xs(kp),
                            start=(kp == 0),
                            stop=(t == 0 and kp == KP - 1),
                            perf_mode=DR,
                        )

                grps = [(r, role) for r in half for role in roles]
                if t == 0 and half == (0, 1):
                    # first 4 groups in two passes: pass 1 needs only the
                    # first x half, so PE starts while the rest streams in
                    for r, role in grps[:4]:
                        emit_group(r, role, range(KH))
                    for r, role in grps[:4]:
                        emit_group(r, role, range(KH, KP))
                    for r, role in grps[4:]:
                        emit_group(r, role, range(KP))
                else:
                    for r, role in grps:
                        emit_group(r, role, range(KP))
                # recurrent matmuls accumulate into the same PSUM groups
                if t > 0:
                    for r in half:
                        for role in roles:
                            m = _chunk(role, r)
                            p = psums[(role, r)]
                            for kp in range(2):
                                nc.tensor.matmul(
                                    p[:],
                                    wh1_sb[:, m * 4 + 2 * kp:m * 4 + 2 * kp + 2, :],
                                    h1p[:, 2 * kp:2 * kp + 2, :],
                                    start=False,
                                    stop=(kp == 1),
                                    perf_mode=DR,
                                )
                # gate nonlinearities + state update per row
                for r in half:
                    si = gate.tile([128, BL], F32, name="si", tag="gt")
                    nc.scalar.activation(si[:], psums[("i", r)][:], Act.Sigmoid,
                                         bias=bias_ap(_chunk("i", r)), scale=INV)
                    tg = gate.tile([128, BL], F32, name="tg", tag="gt")
                    nc.scalar.activation(tg[:], psums[("g", r)][:], Act.Tanh,
                                         bias=bias_ap(_chunk("g", r)), scale=INV)
                    so = gate.tile([128, BL], F32, name="so", tag="gt")
                    nc.scalar.activation(so[:], psums[("o", r)][:], Act.Sigmoid,
                                         bias=bias_ap(_chunk("o", r)), scale=INV)
                    ig = gate.tile([128, BL], F32, name="ig", tag="gt")
                    nc.vector.tensor_tensor(ig[:], si[:], tg[:], Alu.mult)
                    if t == 0:
                        cn = ig
                    else:
                        sf = gate.tile([128, BL], F32, name="sf", tag="gt")
                        nc.scalar.activation(sf[:], psums[("f", r)][:], Act.Sigmoid,
                                             bias=bias_ap(_chunk("f", r)), scale=INV)
                        fc = gate.tile([128, BL], F32, name="fc", tag="gt")
                        nc.vector.tensor_tensor(fc[:], sf[:], c1[r][:], Alu.mult)
                        cn = gate.tile([128, BL], F32, name="cn", tag="gt")
                        nc.vector.tensor_tensor(cn[:], fc[:], ig[:], Alu.add)
                    # c1 = relu(cn) on DVE; h1 = so * relu(tanh(cn))
                    # (== relu(so * tanh(relu(cn))) since so > 0, tanh monotone)
                    nc.vector.tensor_scalar_max(c1[r][:], cn[:], 0.0)
                    tc_ = gate.tile([128, BL], F32, name="tc", tag="gt")
                    nc.scalar.activation(tc_[:], cn[:], Act.Tanh)
                    nc.vector.scalar_tensor_tensor(h1[r][:], tc_[:], 0.0, so[:],
                                                   Alu.max, Alu.mult)

            if t == 1:
                emit_lblock(0, h1)          # L2: h1 @ t1 (pre-overwrite)
            if t == 2:
                emit_lblock(1, h2)

        emit_lblock(2, h3)

        # --- output head: f32r matmul + relu + Bernoulli threshold ------------
        for r in range(NR):
            p = ps.tile([128, BL], F32, name="psum", tag="psum")
            for k in range(NR):
                nc.tensor.matmul(
                    p[:],
                    wo_sb[:, r * 512 + k * 128:r * 512 + (k + 1) * 128],
                    h4[k][:],
                    start=(k == 0),
                    stop=(k == NR - 1),
                )
            probs = gate.tile([128, BL], F32, name="probs", tag="gt")
            nc.scalar.activation(probs[:], p[:], Act.Relu, bias=bias_ap(64 + r))
            act = gate.tile([128, BL], F32, name="act", tag="gt")
            nc.vector.tensor_tensor(act[:], probs[:], u_sb[:, r * BL:(r + 1) * BL], Alu.is_gt)
            nc.sync.dma_start(out_ext[r], act[:])

    nc.finalize()
    return nc


# ---------------------------------------------------------------------------
# Host-side input prep / output assembly
# ---------------------------------------------------------------------------
def _tile_weight(wT: np.ndarray, dtype, scale=1.0) -> np.ndarray:
    """[K, M] (transposed weight) -> [128, M*K/128] where
    arr[p, m*K + k*128 + j] = wT[k*128+p, m*128+j]."""
    K, M = wT.shape
    kc, mc = K // 128, M // 128
    return np.ascontiguousarray(
        (wT * scale).reshape(kc, 128, mc, 128).transpose(1, 2, 0, 3).reshape(128, M * kc)
    ).astype(dtype)


def _tile_weight_w1(wT: np.ndarray, dtype, scale=1.0) -> np.ndarray:
    """[K, M] -> [M/128, 128, K]: arr[m, p, k*128+j] = wT[k*128+p, m*128+j]."""
    K, M = wT.shape
    kc, mc = K // 128, M // 128
    return np.ascontiguousarray(
        (wT * scale).reshape(kc, 128, mc, 128).transpose(2, 1, 0, 3).reshape(mc, 128, K)
    ).astype(dtype)


def prep_core_inputs(inputs: dict) -> list[dict]:
    """Full inputs -> per-core in_maps with host-side transpose/tiling."""
    w1 = _tile_weight_w1(np.ascontiguousarray(inputs["Wih_c1"].T), E4M3,
                         scale=WSCALE).reshape(NM, 128, KX, 128)
    wh1 = _tile_weight(np.ascontiguousarray(inputs["Whh_c1"].T), E4M3,
                       scale=WSCALE).reshape(128, NM * 4, 128)
    w2 = _tile_weight(np.ascontiguousarray(inputs["Wih_c2"].T), BF16)
    w3 = _tile_weight(np.ascontiguousarray(inputs["Wih_c3"].T), BF16)
    w4 = _tile_weight(np.ascontiguousarray(inputs["Wih_c4"].T), BF16)
    wo = _tile_weight(np.ascontiguousarray(inputs["W_out"].T.astype(np.float32)), np.float32)

    bias = np.zeros((128, 68), np.float32)
    for col, name in ((0, "c1"), (16, "c2"), (32, "c3"), (48, "c4")):
        b = (inputs[f"bih_{name}"].astype(np.float32)
             + inputs[f"bhh_{name}"].astype(np.float32))
        bias[:, col:col + 16] = b.reshape(16, 128).T
    bias[:, 64:68] = inputs["b_out"].astype(np.float32).reshape(4, 128).T

    state = np.asarray(inputs["state"], np.float32)
    goal = np.asarray(inputs["goal"], np.float32)
    u = np.asarray(inputs["u"], np.float32)

    in_maps = []
    for c in range(N_CORES):
        sl = slice(c * BL, (c + 1) * BL)
        xc = np.concatenate([state[sl], goal[sl]], axis=-1)       # [BL, T, IN_DIM]
        # [T, 2, 128, KX/2, BL]: xp[t, s, p, kk, b] = xc[b, t, (s*KX/2+kk)*128+p]
        xp = np.ascontiguousarray(
            xc.transpose(1, 2, 0).reshape(T, KX, 128, BL).transpose(0, 2, 1, 3)
            .reshape(T, 128, KX, BL)
        ).astype(E4M3).transpose(0, 2, 1, 3).reshape(T, 2, KX // 2, 128, BL) \
            .transpose(0, 1, 3, 2, 4)
        xp = np.ascontiguousarray(xp)
        # u: [BL, ACT] -> [128, NR*BL]: up[p, r*BL+b] = u[b, r*128+p]
        up = np.ascontiguousarray(
            u[sl].T.reshape(NR, 128, BL).transpose(1, 0, 2).reshape(128, NR * BL),
            dtype=np.float32,
        )
        in_maps.append({
            "x": xp, "w1": w1, "wh1": wh1, "w2": w2, "w3": w3, "w4": w4,
            "wo": wo, "bias": bias, "u": up,
        })
    return in_maps


def assemble_output(results: list[dict]) -> np.ndarray:
    out = np.empty((B, ACT_DIM), np.float32)
    for c in range(N_CORES):
        a = results[c]["out"].reshape(ACT_DIM, BL)    # [acts, batch]
        out[c * BL:(c + 1) * BL] = a.T
    return out


_NC_CACHE = None


def kernel(**inputs) -> np.ndarray:
    global _NC_CACHE
    import os

    from concourse.bass_utils import run_bass_kernel_spmd

    # profiling shims aren't installed here; never let an inherited
    # BASS_TRACE flip run_bass_kernel_spmd into the trace path
    os.environ["BASS_NEVER_TRACE"] = "1"

    inputs = {k: np.asarray(v) for k, v in inputs.items()}
    if _NC_CACHE is None:
        _NC_CACHE = build_kernel()
    in_maps = prep_core_inputs(inputs)
    res = run_bass_kernel_spmd(_NC_CACHE, in_maps, core_ids=list(range(N_CORES)))
    return assemble_output(res.results)


if __name__ == "__main__":
    import reference

    inputs = reference.setup_inputs()
    inputs = {k: np.asarray(v) for k, v in inputs.items()}
    got = kernel(**inputs)
    want = np.asarray(reference.reference(**inputs))
    flips = (got != want).sum()
    print("flips:", int(flips), "rel_err:",
          np.linalg.norm(got - want) / max(np.linalg.norm(want), 1e-30))


# revision 12
# speedup vs baseline: 1.0886x; 1.0886x over previous
"""Trainium2 Bass kernel for nn_Actor (MTRNN actor: 4-step LSTM stack + Bernoulli head).

Data-parallel over 8 NeuronCores: batch 4096 -> 512 rows/core, weights replicated.
Everything on-chip lives in [feature, batch] (transposed) layout; all transposes and
weight tiling happen on the host so the NEFF contains only matmuls + elementwise.

Precision: LSTM1 (x-matmul + recurrent) in fp8 e4m3 operands with DoubleRow perf
mode (2 fp8 MACs per PE cell per cycle) and fp32 PSUM accumulation; weights are
host-scaled by 64 to stay in e4m3 normal range (undone by the activation's scale).
LSTM2-4 stay bf16 and the output head fp32 — fp8 there costs action-bit flips
(validated host-side: this split flips 2 of 2.1M action bits vs fp32, well inside
the rel_err 2e-2 gate).
"""
import sys
from contextlib import ExitStack

import numpy as np

sys.path.insert(0, "/opt/trn_rl_repo")

import ml_dtypes

import concourse.bass as bass
import concourse.tile as tile
from concourse import bacc, mybir
from concourse.vector_clock import ScopedClock

BF16 = ml_dtypes.bfloat16
E4M3 = ml_dtypes.float8_e4m3

H = 512
T = 4
IN_DIM = 3072
ACT_DIM = 512
B = 4096
N_CORES = 8
BL = B // N_CORES          # 512 batch rows per core
KX = IN_DIM // 128         # 24 input-feature chunks
KP = KX // 2               # 12 DoubleRow k-pairs
NM = 16                    # gate chunks (4H/128)
NR = 4                     # H row chunks
WSCALE = 64.0              # host-side weight scale into e4m3 normal range

F32 = mybir.dt.float32
BF = mybir.dt.bfloat16
FP8 = mybir.dt.float8e4
Act = mybir.ActivationFunctionType
Alu = mybir.AluOpType
DR = mybir.MatmulPerfMode.DoubleRow

# w1 gate-chunk pairs per DMA tile, in t0 usage order (i/g/o chunks first,
# f chunks last — they are first needed at t1)
W1PAIRS = ((0, 8), (12, 1), (9, 13), (2, 10), (14, 3), (11, 15), (4, 5), (6, 7))


# ---------------------------------------------------------------------------
# TileContext drain patch: this walrus caps sync-waits per instruction, while
# the stock Tile exit puts one wait per live semaphore on a single Drain.
# Redistribute: one nop per wait, then a wait-free drain.
# ---------------------------------------------------------------------------
def _split_drain_and_barrier(self, tick_clock, wait_clock):
    nc = self.nc
    collector = nc.sync.nop(nofuse=True)
    wait_clock.add_sem_waits(collector.ins, ScopedClock({None: tick_clock.global_clock}))
    si = collector.ins.sync_info
    waits = list(si.on_wait) if si is not None else []
    if len(waits) > 1:
        collector.ins.sync_info = None
        id2sem = {h.num: h for h in self.sems.allocated().values()}
        for w in waits:
            nc.sync.nop(nofuse=True).wait_op(id2sem[w.id], w.wait_value, "sem-ge")
    nc.sync.drain()
    nc.all_engine_barrier()
    assert self.sems is not None
    popped = nc._tile_sem_poison_stack.pop()
    assert popped is self._sem_poison
    nc.clear_and_free_semaphores(list(self.sems.allocated().values()))
    nc.all_engine_barrier()


tile.TileContext._drain_and_barrier = _split_drain_and_barrier


def _chunk(role: str, r: int) -> int:
    """Gate chunk index for role in torch LSTM order [i, f, g, o]."""
    return {"i": 0, "f": 1, "g": 2, "o": 3}[role] * NR + r


def build_kernel() -> bass.Bass:
    nc = bacc.Bacc()

    KH = KP // 2            # 6 k-pairs per x half-tile

    x_ext = nc.declare_dram_parameter("x", [T, 2, 128, 2 * KH, BL], FP8, isOutput=False)
    w1_ext = nc.declare_dram_parameter("w1", [NM // 2, 128, KX, 2, 128], FP8,
                                       isOutput=False)
    wh1_ext = nc.declare_dram_parameter("wh1", [128, NM * 4, 128], FP8, isOutput=False)
    w2_ext = nc.declare_dram_parameter("w2", [128, NM * 512], BF, isOutput=False)
    w3_ext = nc.declare_dram_parameter("w3", [128, NM * 512], BF, isOutput=False)
    w4_ext = nc.declare_dram_parameter("w4", [128, NM * 512], BF, isOutput=False)
    wo_ext = nc.declare_dram_parameter("wo", [128, NR * 512], F32, isOutput=False)
    bias_ext = nc.declare_dram_parameter("bias", [128, 68], F32, isOutput=False)
    u_ext = nc.declare_dram_parameter("u", [128, NR * BL], F32, isOutput=False)
    out_ext = nc.declare_dram_parameter("out", [NR, 128, BL], F32, isOutput=True)

    with ExitStack() as ctx:
        tc = ctx.enter_context(tile.TileContext(nc))
        pers = ctx.enter_context(tc.tile_pool(name="pers", bufs=1))
        gate = ctx.enter_context(tc.tile_pool(name="gate", bufs=12))
        ps = ctx.enter_context(tc.tile_pool(name="ps", bufs=8, space="PSUM"))

        def load_x_step(t):
            tiles = []
            for s in range(2):
                xt = pers.tile([128, 2 * KH, BL], FP8, name=f"x_t{t}_{s}",
                               tag=f"xh{s}", bufs=2)
                # t0 halves go down the sync HWDGE ring (starts earliest);
                # later steps stay on the gpsimd SWDGE ring, which starts
                # slowest and must never gate a t0-critical semaphore
                eng = nc.sync if t == 0 else nc.gpsimd
                eng.dma_start(xt[:], x_ext[t][s])
                tiles.append(xt)
            return tiles

        x_step = load_x_step(0)
        bias_sb = pers.tile([128, 68], F32, name="bias", tag="bias")
        nc.gpsimd.dma_start(bias_sb[:], bias_ext[:])
        w1_tiles = {}           # gate-chunk m -> (tile, local slot)

        def load_w1(g, eng):
            w1t = pers.tile([128, KX, 2, 128], FP8, name=f"w1p_{g}", tag=f"w1p_{g}")
            eng.dma_start(w1t[:], w1_ext[g])
            for ml, m in enumerate(W1PAIRS[g]):
                w1_tiles[m] = (w1t, ml)

        # w1 as 8 double-chunk tiles (6KB/partition descriptors — the HW rings
        # crawl at ~85GB/s on 3KB lines but >~200GB/s on larger ones), in t0
        # usage order: the 12 i/g/o chunks stream on the scalar ring, the 4 f
        # chunks (first needed at t1) go to gpsimd behind bias
        for g in range(6):
            load_w1(g, nc.scalar)
        for g in (6, 7):
            load_w1(g, nc.gpsimd)
        warm = pers.tile([128, 1], F32, name="warm", tag="warm")
        nc.scalar.activation(warm[:], bias_sb[:, 0:1], Act.Sigmoid)
        nc.scalar.activation(warm[:], bias_sb[:, 0:1], Act.Tanh)
        dmy_w = pers.tile([128, 128], BF, name="dmy_w", tag="dmy_w")
        nc.vector.memset(dmy_w[:], 0.0)
        dmy_x = pers.tile([128, BL], BF, name="dmy_x", tag="dmy_x")
        nc.vector.memset(dmy_x[:], 0.0)
        dmy_p = ps.tile([128, BL], F32, name="dmy_p", tag="psum")
        for _ in range(12):
            nc.tensor.matmul(dmy_p[:], dmy_w[:], dmy_x[:], start=True, stop=True)
        nc.scalar.activation(warm[:], dmy_p[:, 0:1], Act.Relu)
        wh1_sb = pers.tile([128, NM * 4, 128], FP8, name="wh1", tag="wh1")
        nc.gpsimd.dma_start(wh1_sb[:], wh1_ext[:])

        # persistent state
        c1 = [pers.tile([128, BL], F32, name=f"c1_{r}", tag=f"c1_{r}") for r in range(NR)]
        h1 = [pers.tile([128, BL], BF, name=f"h1_{r}", tag=f"h1_{r}") for r in range(NR)]
        h1p = pers.tile([128, NR, BL], FP8, name="h1p", tag="h1p")
        h2 = [pers.tile([128, BL], BF, name=f"h2_{r}", tag=f"h2_{r}") for r in range(NR)]
        h3 = [pers.tile([128, BL], BF, name=f"h3_{r}", tag=f"h3_{r}") for r in range(NR)]
        h4 = []
        wl_tiles = {}
        u_sb = None
        INV = 1.0 / WSCALE

        def bias_ap(col):
            return bias_sb[:, col:col + 1]

        def emit_lblock(idx, src_h):
            wl = wl_tiles[("w2", "w3", "w4")[idx]]
            if idx == 2 and not h4:
                h4.extend(
                    pers.tile([128, BL], mybir.dt.float32r, name=f"h4_{r}", tag=f"h4_{r}")
                    for r in range(NR)
                )
            dst = (h2, h3, h4)[idx]
            bias_off = 16 * (idx + 1)
            for half in ((0, 1), (2, 3)):
                psums = {}
                for r in half:
                    for role in ("i", "g", "o"):     # f-gate unused (c_prev=0)
                        m = _chunk(role, r)
                        p = ps.tile([128, BL], F32, name="psum", tag="psum")
                        psums[(role, r)] = p
                        for k in range(NR):
                            nc.tensor.matmul(
                                p[:],
                                wl[:, m * 512 + k * 128:m * 512 + (k + 1) * 128],
                                src_h[k][:],
                                start=(k == 0),
                                stop=(k == NR - 1),
                            )
                for r in half:
                    si = gate.tile([128, BL], F32, name="si", tag="gt")
                    nc.scalar.activation(si[:], psums[("i", r)][:], Act.Sigmoid,
                                         bias=bias_ap(bias_off + _chunk("i", r)))
                    tg = gate.tile([128, BL], F32, name="tg", tag="gt")
                    nc.scalar.activation(tg[:], psums[("g", r)][:], Act.Tanh,
                                         bias=bias_ap(bias_off + _chunk("g", r)))
                    so = gate.tile([128, BL], F32, name="so", tag="gt")
                    nc.scalar.activation(so[:], psums[("o", r)][:], Act.Sigmoid,
                                         bias=bias_ap(bias_off + _chunk("o", r)))
                    cn = gate.tile([128, BL], F32, name="ig", tag="gt")
                    nc.vector.tensor_tensor(cn[:], si[:], tg[:], Alu.mult)
                    tc_ = gate.tile([128, BL], F32, name="tc", tag="gt")
                    nc.scalar.activation(tc_[:], cn[:], Act.Tanh)
                    nc.vector.scalar_tensor_tensor(dst[r][:], tc_[:], 0.0, so[:],
                                                   Alu.max, Alu.mult)

        # --- LSTM1: T fused steps (fp8 DoubleRow matmuls) ---------------------
        for t in range(T):
            roles = ("i", "g", "o") if t == 0 else ("i", "f", "g", "o")
            x_cur = x_step
            if t + 1 < T:
                x_step = load_x_step(t + 1)
            # prefetch the L-block weight for this step's tail (w2/w3/w4),
            # plus head tensors at the last step
            if t >= 1:
                name = ("w2", "w3", "w4")[t - 1]
                wlt = pers.tile([128, NM * 512], BF, name=name, tag="wl", bufs=2)
                nc.gpsimd.dma_start(wlt[:], (w2_ext, w3_ext, w4_ext)[t - 1][:])
                wl_tiles[name] = wlt
            if t > 0:
                for r in range(NR):
                    nc.vector.tensor_copy(h1p[:, r, :], h1[r][:])
            if t == T - 1:
                wo_sb = pers.tile([128, NR * 512], mybir.dt.float32r, name="wo", tag="wo")
                nc.gpsimd.dma_start(wo_sb[:], wo_ext[:])
                u_sb = pers.tile([128, NR * BL], F32, name="u", tag="u")
                nc.gpsimd.dma_start(u_sb[:], u_ext[:])

            for half in ((0, 1), (2, 3)):
                psums = {}

                def xs(kp):
                    # rhs pair kp (0..KP-1): [128, 2, BL] slice of an x half-tile
                    s, kk = kp // KH, kp % KH
                    return x_cur[s][:, 2 * kk:2 * kk + 2, :]

                def emit_group(r, role, kps):
                    m = _chunk(role, r)
                    if (role, r) not in psums:
                        psums[(role, r)] = ps.tile([128, BL], F32,
                                                   name="psum", tag="psum")
                    p = psums[(role, r)]
                    w1t, ml = w1_tiles[m]
                    for kp in kps:
                        nc.tensor.matmul(
                            p[:],
                            w1t[:, 2 * kp:2 * kp + 2, ml, :],
                            xs(kp),
                            start=(kp == 0),
                            stop=(t == 0 and kp == KP - 1),
                            perf_mode=DR,
                        )

                grps = [(r, role) for r in half for role in roles]
                if t == 0 and half == (0, 1):
                    # first 4 groups in two passes: pass 1 needs only the
                    # first x half, so PE starts while the rest streams in
                    for r, role in grps[:4]:
                        emit_group(r, role, range(KH))
                    for r, role in grps[:4]:
                        emit_group(r, role, range(KH, KP))
                    for r, role in grps[4:]:
                        emit_group(r, role, range(KP))
                else:
                    for r, role in grps:
                        emit_group(r, role, range(KP))
                # recurrent matmuls accumulate into the same PSUM groups
                if t > 0:
                    for r in half:
                        for role in roles:
                            m = _chunk(role, r)
                            p = psums[(role, r)]
                            for kp in range(2):
                                nc.tensor.matmul(
                                    p[:],
                                    wh1_sb[:, m * 4 + 2 * kp:m * 4 + 2 * kp + 2, :],
                                    h1p[:, 2 * kp:2 * kp + 2, :],
                                    start=False,
                                    stop=(kp == 1),
                                    perf_mode=DR,
                                )
                # gate nonlinearities + state update per row
                for r in half:
                    si = gate.tile([128, BL], F32, name="si", tag="gt")
                    nc.scalar.activation(si[:], psums[("i", r)][:], Act.Sigmoid,
                                         bias=bias_ap(_chunk("i", r)), scale=INV)
                    tg = gate.tile([128, BL], F32, name="tg", tag="gt")
                    nc.scalar.activation(tg[:], psums[("g", r)][:], Act.Tanh,
                                         bias=bias_ap(_chunk("g", r)), scale=INV)
                    so = gate.tile([128, BL], F32, name="so", tag="gt")
                    nc.scalar.activation(so[:], psums[("o", r)][:], Act.Sigmoid,
                                         bias=bias_ap(_chunk("o", r)), scale=INV)
                    ig = gate.tile([128, BL], F32, name="ig", tag="gt")
                    nc.vector.tensor_tensor(ig[:], si[:], tg[:], Alu.mult)
                    if t == 0:
                        cn = ig
                    else:
                        sf = gate.tile([128, BL], F32, name="sf", tag="gt")
                        nc.scalar.activation(sf[:], psums[("f", r)][:], Act.Sigmoid,
                                             bias=bias_ap(_chunk("f", r)), scale=INV)
                        fc = gate.tile([128, BL], F32, name="fc", tag="gt")
                        nc.vector.tensor_tensor(fc[:], sf[:], c1[r][:], Alu.mult)
                        cn = gate.tile([128, BL], F32, name="cn", tag="gt")
                        nc.vector.tensor_tensor(cn[:], fc[:], ig[:], Alu.add)
                    # c1 = relu(cn) on DVE; h1 = so * relu(tanh(cn))
                    # (== relu(so * tanh(relu(cn))) since so > 0, tanh monotone)
                    nc.vector.tensor_scalar_max(c1[r][:], cn[:], 0.0)
                    tc_ = gate.tile([128, BL], F32, name="tc", tag="gt")
                    nc.scalar.activation(tc_[:], cn[:], Act.Tanh)
                    nc.vector.scalar_tensor_tensor(h1[r][:], tc_[:], 0.0, so[:],
                                                   Alu.max, Alu.mult)

            if t == 1:
                emit_lblock(0, h1)          # L2: h1 @ t1 (pre-overwrite)
            if t == 2:
                emit_lblock(1, h2)

        emit_lblock(2, h3)

        # --- output head: f32r matmul + relu + Bernoulli threshold ------------
        for r in range(NR):
            p = ps.tile([128, BL], F32, name="psum", tag="psum")
            for k in range(NR):
                nc.tensor.matmul(
                    p[:],
                    wo_sb[:, r * 512 + k * 128:r * 512 + (k + 1) * 128],
                    h4[k][:],
                    start=(k == 0),
                    stop=(k == NR - 1),
                )
            probs = gate.tile([128, BL], F32, name="probs", tag="gt")
            nc.scalar.activation(probs[:], p[:], Act.Relu, bias=bias_ap(64 + r))
            act = gate.tile([128, BL], F32, name="act", tag="gt")
            nc.vector.tensor_tensor(act[:], probs[:], u_sb[:, r * BL:(r + 1) * BL], Alu.is_gt)
            nc.sync.dma_start(out_ext[r], act[:])

    nc.finalize()
    return nc


# ---------------------------------------------------------------------------
# Host-side input prep / output assembly
# ---------------------------------------------------------------------------
def _tile_weight(wT: np.ndarray, dtype, scale=1.0) -> np.ndarray:
    """[K, M] (transposed weight) -> [128, M*K/128] where
    arr[p, m*K + k*128 + j] = wT[k*128+p, m*128+j]."""
    K, M = wT.shape
    kc, mc = K // 128, M // 128
    return np.ascontiguousarray(
        (wT * scale).reshape(kc, 128, mc, 128).transpose(1, 2, 0, 3).reshape(128, M * kc)
    ).astype(dtype)


def _tile_weight_w1(wT: np.ndarray, dtype, scale=1.0) -> np.ndarray:
    """[K, M] -> [M/128, 128, K]: arr[m, p, k*128+j] = wT[k*128+p, m*128+j]."""
    K, M = wT.shape
    kc, mc = K // 128, M // 128
    return np.ascontiguousarray(
        (wT * scale).reshape(kc, 128, mc, 128).transpose(2, 1, 0, 3).reshape(mc, 128, K)
    ).astype(dtype)


def prep_core_inputs(inputs: dict) -> list[dict]:
    """Full inputs -> per-core in_maps with host-side transpose/tiling."""
    w1_rs = _tile_weight_w1(np.ascontiguousarray(inputs["Wih_c1"].T), E4M3,
                            scale=WSCALE).reshape(NM, 128, KX, 128)
    # pack gate-chunk pairs per W1PAIRS: [8, 128, KX, 2, 128]
    w1 = np.ascontiguousarray(w1_rs[np.array(W1PAIRS)].transpose(0, 2, 3, 1, 4))
    wh1 = _tile_weight(np.ascontiguousarray(inputs["Whh_c1"].T), E4M3,
                       scale=WSCALE).reshape(128, NM * 4, 128)
    w2 = _tile_weight(np.ascontiguousarray(inputs["Wih_c2"].T), BF16)
    w3 = _tile_weight(np.ascontiguousarray(inputs["Wih_c3"].T), BF16)
    w4 = _tile_weight(np.ascontiguousarray(inputs["Wih_c4"].T), BF16)
    wo = _tile_weight(np.ascontiguousarray(inputs["W_out"].T.astype(np.float32)), np.float32)

    bias = np.zeros((128, 68), np.float32)
    for col, name in ((0, "c1"), (16, "c2"), (32, "c3"), (48, "c4")):
        b = (inputs[f"bih_{name}"].astype(np.float32)
             + inputs[f"bhh_{name}"].astype(np.float32))
        bias[:, col:col + 16] = b.reshape(16, 128).T
    bias[:, 64:68] = inputs["b_out"].astype(np.float32).reshape(4, 128).T

    state = np.asarray(inputs["state"], np.float32)
    goal = np.asarray(inputs["goal"], np.float32)
    u = np.asarray(inputs["u"], np.float32)

    in_maps = []
    for c in range(N_CORES):
        sl = slice(c * BL, (c + 1) * BL)
        xc = np.concatenate([state[sl], goal[sl]], axis=-1)       # [BL, T, IN_DIM]
        # [T, 2, 128, KX/2, BL]: xp[t, s, p, kk, b] = xc[b, t, (s*KX/2+kk)*128+p]
        xp = np.ascontiguousarray(
            xc.transpose(1, 2, 0).reshape(T, KX, 128, BL).transpose(0, 2, 1, 3)
            .reshape(T, 128, KX, BL)
        ).astype(E4M3).transpose(0, 2, 1, 3).reshape(T, 2, KX // 2, 128, BL) \
            .transpose(0, 1, 3, 2, 4)
        xp = np.ascontiguousarray(xp)
        # u: [BL, ACT] -> [128, NR*BL]: up[p, r*BL+b] = u[b, r*128+p]
        up = np.ascontiguousarray(
            u[sl].T.reshape(NR, 128, BL).transpose(1, 0, 2).reshape(128, NR * BL),
            dtype=np.float32,
        )
        in_maps.append({
            "x": xp, "w1": w1, "wh1": wh1, "w2": w2, "w3": w3, "w4": w4,
            "wo": wo, "bias": bias, "u": up,
        })
    return in_maps


def assemble_output(results: list[dict]) -> np.ndarray:
    out = np.empty((B, ACT_DIM), np.float32)
    for c in range(N_CORES):
        a = results[c]["out"].reshape(ACT_DIM, BL)    # [acts, batch]
        out[c * BL:(c + 1) * BL] = a.T
    return out


_NC_CACHE = None


def kernel(**inputs) -> np.ndarray:
    global _NC_CACHE
    import os

    from concourse.bass_utils import run_bass_kernel_spmd

    # profiling shims aren't installed here; never let an inherited
    # BASS_TRACE flip run_bass_kernel_spmd into the trace path
    os.environ["BASS_NEVER_TRACE"] = "1"

    inputs = {k: np.asarray(v) for k, v in inputs.items()}
    if _NC_CACHE is None:
        _NC_CACHE = build_kernel()
    in_maps = prep_core_inputs(inputs)
    res = run_bass_kernel_spmd(_NC_CACHE, in_maps, core_ids=list(range(N_CORES)))
    return assemble_output(res.results)


if __name__ == "__main__":
    import reference

    inputs = reference.setup_inputs()
    inputs = {k: np.asarray(v) for k, v in inputs.items()}
    got = kernel(**inputs)
    want = np.asarray(reference.reference(**inputs))
    flips = (got != want).sum()
    print("flips:", int(flips), "rel_err:",
          np.linalg.norm(got - want) / max(np.linalg.norm(want), 1e-30))


# revision 20
# speedup vs baseline: 1.0907x; 1.0019x over previous
"""Trainium2 Bass kernel for nn_Actor (MTRNN actor: 4-step LSTM stack + Bernoulli head).

Data-parallel over 8 NeuronCores: batch 4096 -> 512 rows/core, weights replicated.
Everything on-chip lives in [feature, batch] (transposed) layout; all transposes and
weight tiling happen on the host so the NEFF contains only matmuls + elementwise.

Precision: LSTM1 (x-matmul + recurrent) in fp8 e4m3 operands with DoubleRow perf
mode (2 fp8 MACs per PE cell per cycle) and fp32 PSUM accumulation; weights are
host-scaled by 64 to stay in e4m3 normal range (undone by the activation's scale).
LSTM2-4 stay bf16 and the output head fp32 — fp8 there costs action-bit flips
(validated host-side: this split flips 2 of 2.1M action bits vs fp32, well inside
the rel_err 2e-2 gate).
"""
import sys
from contextlib import ExitStack

import numpy as np

sys.path.insert(0, "/opt/trn_rl_repo")

import ml_dtypes

import concourse.bass as bass
import concourse.tile as tile
from concourse import bacc, mybir
from concourse.vector_clock import ScopedClock

BF16 = ml_dtypes.bfloat16
E4M3 = ml_dtypes.float8_e4m3

H = 512
T = 4
IN_DIM = 3072
ACT_DIM = 512
B = 4096
N_CORES = 8
BL = B // N_CORES          # 512 batch rows per core
KX = IN_DIM // 128         # 24 input-feature chunks
KP = KX // 2               # 12 DoubleRow k-pairs
NM = 16                    # gate chunks (4H/128)
NR = 4                     # H row chunks
WSCALE = 64.0              # host-side weight scale into e4m3 normal range

F32 = mybir.dt.float32
BF = mybir.dt.bfloat16
FP8 = mybir.dt.float8e4
Act = mybir.ActivationFunctionType
Alu = mybir.AluOpType
DR = mybir.MatmulPerfMode.DoubleRow

# w1 gate-chunk pairs per DMA tile, in t0 usage order (i/g/o chunks first,
# f chunks last — they are first needed at t1)
W1PAIRS = ((0, 8), (12, 1), (9, 13), (2, 10), (14, 3), (11, 15), (4, 5), (6, 7))


# ---------------------------------------------------------------------------
# TileContext drain patch: this walrus caps sync-waits per instruction, while
# the stock Tile exit puts one wait per live semaphore on a single Drain.
# Redistribute: one nop per wait, then a wait-free drain.
# ---------------------------------------------------------------------------
def _split_drain_and_barrier(self, tick_clock, wait_clock):
    nc = self.nc
    collector = nc.sync.nop(nofuse=True)
    wait_clock.add_sem_waits(collector.ins, ScopedClock({None: tick_clock.global_clock}))
    si = collector.ins.sync_info
    waits = list(si.on_wait) if si is not None else []
    if len(waits) > 1:
        collector.ins.sync_info = None
        id2sem = {h.num: h for h in self.sems.allocated().values()}
        for w in waits:
            nc.sync.nop(nofuse=True).wait_op(id2sem[w.id], w.wait_value, "sem-ge")
    nc.sync.drain()
    nc.all_engine_barrier()
    assert self.sems is not None
    popped = nc._tile_sem_poison_stack.pop()
    assert popped is self._sem_poison
    nc.clear_and_free_semaphores(list(self.sems.allocated().values()))
    nc.all_engine_barrier()


tile.TileContext._drain_and_barrier = _split_drain_and_barrier


def _chunk(role: str, r: int) -> int:
    """Gate chunk index for role in torch LSTM order [i, f, g, o]."""
    return {"i": 0, "f": 1, "g": 2, "o": 3}[role] * NR + r


def build_kernel() -> bass.Bass:
    nc = bacc.Bacc()

    KH = KP // 2            # 6 k-pairs per x half-tile

    x_ext = nc.declare_dram_parameter("x", [T, 2, 128, 2 * KH, BL], FP8, isOutput=False)
    w1_ext = nc.declare_dram_parameter("w1", [NM // 2, 128, KX, 2, 128], FP8,
                                       isOutput=False)
    wh1_ext = nc.declare_dram_parameter("wh1", [128, NM * 4, 128], FP8, isOutput=False)
    w2_ext = nc.declare_dram_parameter("w2", [128, NM * 512], BF, isOutput=False)
    w3_ext = nc.declare_dram_parameter("w3", [128, NM * 512], BF, isOutput=False)
    w4_ext = nc.declare_dram_parameter("w4", [128, NM * 512], BF, isOutput=False)
    wo_ext = nc.declare_dram_parameter("wo", [128, NR * 512], F32, isOutput=False)
    bias_ext = nc.declare_dram_parameter("bias", [128, 68], F32, isOutput=False)
    u_ext = nc.declare_dram_parameter("u", [128, NR * BL], F32, isOutput=False)
    out_ext = nc.declare_dram_parameter("out", [NR, 128, BL], F32, isOutput=True)

    with ExitStack() as ctx:
        tc = ctx.enter_context(tile.TileContext(nc))
        pers = ctx.enter_context(tc.tile_pool(name="pers", bufs=1))
        gate = ctx.enter_context(tc.tile_pool(name="gate", bufs=12))
        ps = ctx.enter_context(tc.tile_pool(name="ps", bufs=8, space="PSUM"))

        def load_x_step(t):
            tiles = []
            for s in range(2):
                xt = pers.tile([128, 2 * KH, BL], FP8, name=f"x_t{t}_{s}",
                               tag=f"xh{s}", bufs=2)
                # t0 halves go down the sync HWDGE ring (starts earliest);
                # later steps stay on the gpsimd SWDGE ring, which starts
                # slowest and must never gate a t0-critical semaphore
                eng = nc.sync if t == 0 else nc.gpsimd
                eng.dma_start(xt[:], x_ext[t][s])
                tiles.append(xt)
            return tiles

        x_step = load_x_step(0)
        bias_sb = pers.tile([128, 68], F32, name="bias", tag="bias")
        nc.gpsimd.dma_start(bias_sb[:], bias_ext[:])
        # single w1 tile (one semaphore), filled by 8 double-chunk DMAs
        # (6KB/partition descriptors — the HW rings crawl at ~85GB/s on 3KB
        # lines but >~200GB/s on larger ones), in t0 usage order: the 12
        # i/g/o chunks stream on the scalar ring, the 4 f chunks (first
        # needed at t1) go to gpsimd behind bias
        w1_sb = pers.tile([128, NM // 2, KX, 2, 128], FP8, name="w1", tag="w1")
        w1_slot = {}            # gate-chunk m -> (pair idx, local slot)
        for g, pair in enumerate(W1PAIRS):
            eng = nc.scalar if g < 6 else nc.gpsimd
            eng.dma_start(w1_sb[:, g], w1_ext[g])
            for ml, m in enumerate(pair):
                w1_slot[m] = (g, ml)
        warm = pers.tile([128, 1], F32, name="warm", tag="warm")
        nc.scalar.activation(warm[:], bias_sb[:, 0:1], Act.Sigmoid)
        nc.scalar.activation(warm[:], bias_sb[:, 0:1], Act.Tanh)
        dmy = pers.tile([128, BL], BF, name="dmy", tag="dmy")
        nc.vector.memset(dmy[:], 0.0)
        dmy_p = ps.tile([128, BL], F32, name="dmy_p", tag="psum")
        for _ in range(12):
            nc.tensor.matmul(dmy_p[:], dmy[:, 0:128], dmy[:], start=True, stop=True)
        nc.scalar.activation(warm[:], dmy_p[:, 0:1], Act.Relu)
        wh1_sb = pers.tile([128, NM * 4, 128], FP8, name="wh1", tag="wh1")
        nc.gpsimd.dma_start(wh1_sb[:], wh1_ext[:])

        # persistent state, one tile per layer (fewer tile semaphores -> a
        # shorter final drain wall)
        c1 = pers.tile([128, NR, BL], F32, name="c1", tag="c1")
        h1 = pers.tile([128, NR, BL], BF, name="h1", tag="h1")
        h1p = pers.tile([128, NR, BL], FP8, name="h1p", tag="h1p")
        h2 = pers.tile([128, NR, BL], BF, name="h2", tag="h2")
        h3 = pers.tile([128, NR, BL], BF, name="h3", tag="h3")
        h4 = pers.tile([128, NR, BL], mybir.dt.float32r, name="h4", tag="h4")
        wl_tiles = {}
        u_sb = None
        INV = 1.0 / WSCALE

        def bias_ap(col):
            return bias_sb[:, col:col + 1]

        def emit_lblock(idx, src_h):
            wl = wl_tiles[("w2", "w3", "w4")[idx]]
            dst = (h2, h3, h4)[idx]
            bias_off = 16 * (idx + 1)
            for half in ((0, 1), (2, 3)):
                psums = {}
                for r in half:
                    for role in ("i", "g", "o"):     # f-gate unused (c_prev=0)
                        m = _chunk(role, r)
                        p = ps.tile([128, BL], F32, name="psum", tag="psum")
                        psums[(role, r)] = p
                        for k in range(NR):
                            nc.tensor.matmul(
                                p[:],
                                wl[:, m * 512 + k * 128:m * 512 + (k + 1) * 128],
                                src_h[:, k, :],
                                start=(k == 0),
                                stop=(k == NR - 1),
                            )
                for r in half:
                    si = gate.tile([128, BL], F32, name="si", tag="gt")
                    nc.scalar.activation(si[:], psums[("i", r)][:], Act.Sigmoid,
                                         bias=bias_ap(bias_off + _chunk("i", r)))
                    tg = gate.tile([128, BL], F32, name="tg", tag="gt")
                    nc.scalar.activation(tg[:], psums[("g", r)][:], Act.Tanh,
                                         bias=bias_ap(bias_off + _chunk("g", r)))
                    so = gate.tile([128, BL], F32, name="so", tag="gt")
                    nc.scalar.activation(so[:], psums[("o", r)][:], Act.Sigmoid,
                                         bias=bias_ap(bias_off + _chunk("o", r)))
                    cn = gate.tile([128, BL], F32, name="ig", tag="gt")
                    nc.vector.tensor_tensor(cn[:], si[:], tg[:], Alu.mult)
                    tc_ = gate.tile([128, BL], F32, name="tc", tag="gt")
                    nc.scalar.activation(tc_[:], cn[:], Act.Tanh)
                    nc.vector.scalar_tensor_tensor(dst[:, r, :], tc_[:], 0.0, so[:],
                                                   Alu.max, Alu.mult)

        # --- LSTM1: T fused steps (fp8 DoubleRow matmuls) ---------------------
        for t in range(T):
            roles = ("i", "g", "o") if t == 0 else ("i", "f", "g", "o")
            x_cur = x_step
            if t + 1 < T:
                x_step = load_x_step(t + 1)
            # prefetch the L-block weight for this step's tail (w2/w3/w4),
            # plus head tensors at the last step
            if t >= 1:
                name = ("w2", "w3", "w4")[t - 1]
                wlt = pers.tile([128, NM * 512], BF, name=name, tag="wl", bufs=2)
                nc.gpsimd.dma_start(wlt[:], (w2_ext, w3_ext, w4_ext)[t - 1][:])
                wl_tiles[name] = wlt
            if t > 0:
                for r in range(NR):
                    nc.vector.tensor_copy(h1p[:, r, :], h1[:, r, :])
            if t == T - 1:
                wo_sb = pers.tile([128, NR * 512], mybir.dt.float32r, name="wo", tag="wo")
                nc.gpsimd.dma_start(wo_sb[:], wo_ext[:])
                u_sb = pers.tile([128, NR * BL], F32, name="u", tag="u")
                nc.gpsimd.dma_start(u_sb[:], u_ext[:])

            for half in ((0, 1), (2, 3)):
                psums = {}

                def xs(kp):
                    # rhs pair kp (0..KP-1): [128, 2, BL] slice of an x half-tile
                    s, kk = kp // KH, kp % KH
                    return x_cur[s][:, 2 * kk:2 * kk + 2, :]

                def emit_group(r, role, kps):
                    m = _chunk(role, r)
                    if (role, r) not in psums:
                        psums[(role, r)] = ps.tile([128, BL], F32,
                                                   name="psum", tag="psum")
                    p = psums[(role, r)]
                    g, ml = w1_slot[m]
                    for kp in kps:
                        nc.tensor.matmul(
                            p[:],
                            w1_sb[:, g, 2 * kp:2 * kp + 2, ml, :],
                            xs(kp),
                            start=(kp == 0),
                            stop=(t == 0 and kp == KP - 1),
                            perf_mode=DR,
                        )

                grps = [(r, role) for r in half for role in roles]
                if t == 0 and half == (0, 1):
                    # first 4 groups in two passes: pass 1 needs only the
                    # first x half, so PE starts while the rest streams in
                    for r, role in grps[:4]:
                        emit_group(r, role, range(KH))
                    for r, role in grps[:4]:
                        emit_group(r, role, range(KH, KP))
                    for r, role in grps[4:]:
                        emit_group(r, role, range(KP))
                else:
                    for r, role in grps:
                        emit_group(r, role, range(KP))
                # recurrent matmuls accumulate into the same PSUM groups
                if t > 0:
                    for r in half:
                        for role in roles:
                            m = _chunk(role, r)
                            p = psums[(role, r)]
                            for kp in range(2):
                                nc.tensor.matmul(
                                    p[:],
                                    wh1_sb[:, m * 4 + 2 * kp:m * 4 + 2 * kp + 2, :],
                                    h1p[:, 2 * kp:2 * kp + 2, :],
                                    start=False,
                                    stop=(kp == 1),
                                    perf_mode=DR,
                                )
                # gate nonlinearities + state update per row
                for r in half:
                    si = gate.tile([128, BL], F32, name="si", tag="gt")
                    nc.scalar.activation(si[:], psums[("i", r)][:], Act.Sigmoid,
                                         bias=bias_ap(_chunk("i", r)), scale=INV)
                    tg = gate.tile([128, BL], F32, name="tg", tag="gt")
                    nc.scalar.activation(tg[:], psums[("g", r)][:], Act.Tanh,
                                         bias=bias_ap(_chunk("g", r)), scale=INV)
                    so = gate.tile([128, BL], F32, name="so", tag="gt")
                    nc.scalar.activation(so[:], psums[("o", r)][:], Act.Sigmoid,
                                         bias=bias_ap(_chunk("o", r)), scale=INV)
                    ig = gate.tile([128, BL], F32, name="ig", tag="gt")
                    nc.vector.tensor_tensor(ig[:], si[:], tg[:], Alu.mult)
                    if t == 0:
                        cn = ig
                    else:
                        sf = gate.tile([128, BL], F32, name="sf", tag="gt")
                        nc.scalar.activation(sf[:], psums[("f", r)][:], Act.Sigmoid,
                                             bias=bias_ap(_chunk("f", r)), scale=INV)
                        fc = gate.tile([128, BL], F32, name="fc", tag="gt")
                        nc.vector.tensor_tensor(fc[:], sf[:], c1[:, r, :], Alu.mult)
                        cn = gate.tile([128, BL], F32, name="cn", tag="gt")
                        nc.vector.tensor_tensor(cn[:], fc[:], ig[:], Alu.add)
                    # c1 = relu(cn) on DVE; h1 = so * relu(tanh(cn))
                    # (== relu(so * tanh(relu(cn))) since so > 0, tanh monotone)
                    nc.vector.tensor_scalar_max(c1[:, r, :], cn[:], 0.0)
                    tc_ = gate.tile([128, BL], F32, name="tc", tag="gt")
                    nc.scalar.activation(tc_[:], cn[:], Act.Tanh)
                    nc.vector.scalar_tensor_tensor(h1[:, r, :], tc_[:], 0.0, so[:],
                                                   Alu.max, Alu.mult)

            if t == 1:
                emit_lblock(0, h1)          # L2: h1 @ t1 (pre-overwrite)
            if t == 2:
                emit_lblock(1, h2)

        emit_lblock(2, h3)

        # --- output head: f32r matmul + relu + Bernoulli threshold ------------
        for r in range(NR):
            p = ps.tile([128, BL], F32, name="psum", tag="psum")
            for k in range(NR):
                nc.tensor.matmul(
                    p[:],
                    wo_sb[:, r * 512 + k * 128:r * 512 + (k + 1) * 128],
                    h4[:, k, :],
                    start=(k == 0),
                    stop=(k == NR - 1),
                )
            probs = gate.tile([128, BL], F32, name="probs", tag="gt")
            nc.scalar.activation(probs[:], p[:], Act.Relu, bias=bias_ap(64 + r))
            act = gate.tile([128, BL], F32, name="act", tag="gt")
            nc.vector.tensor_tensor(act[:], probs[:], u_sb[:, r * BL:(r + 1) * BL], Alu.is_gt)
            nc.sync.dma_start(out_ext[r], act[:])

    nc.finalize()
    return nc


# ---------------------------------------------------------------------------
# Host-side input prep / output assembly
# ---------------------------------------------------------------------------
def _tile_weight(wT: np.ndarray, dtype, scale=1.0) -> np.ndarray:
    """[K, M] (transposed weight) -> [128, M*K/128] where
    arr[p, m*K + k*128 + j] = wT[k*128+p, m*128+j]."""
    K, M = wT.shape
    kc, mc = K // 128, M // 128
    return np.ascontiguousarray(
        (wT * scale).reshape(kc, 128, mc, 128).transpose(1, 2, 0, 3).reshape(128, M * kc)
    ).astype(dtype)


def _tile_weight_w1(wT: np.ndarray, dtype, scale=1.0) -> np.ndarray:
    """[K, M] -> [M/128, 128, K]: arr[m, p, k*128+j] = wT[k*128+p, m*128+j]."""
    K, M = wT.shape
    kc, mc = K // 128, M // 128
    return np.ascontiguousarray(
        (wT * scale).reshape(kc, 128, mc, 128).transpose(2, 1, 0, 3).reshape(mc, 128, K)
    ).astype(dtype)


def prep_core_inputs(inputs: dict) -> list[dict]:
    """Full inputs -> per-core in_maps with host-side transpose/tiling."""
    w1_rs = _tile_weight_w1(np.ascontiguousarray(inputs["Wih_c1"].T), E4M3,
                            scale=WSCALE).reshape(NM, 128, KX, 128)
    # pack gate-chunk pairs per W1PAIRS: [8, 128, KX, 2, 128]
    w1 = np.ascontiguousarray(w1_rs[np.array(W1PAIRS)].transpose(0, 2, 3, 1, 4))
    wh1 = _tile_weight(np.ascontiguousarray(inputs["Whh_c1"].T), E4M3,
                       scale=WSCALE).reshape(128, NM * 4, 128)
    w2 = _tile_weight(np.ascontiguousarray(inputs["Wih_c2"].T), BF16)
    w3 = _tile_weight(np.ascontiguousarray(inputs["Wih_c3"].T), BF16)
    w4 = _tile_weight(np.ascontiguousarray(inputs["Wih_c4"].T), BF16)
    wo = _tile_weight(np.ascontiguousarray(inputs["W_out"].T.astype(np.float32)), np.float32)

    bias = np.zeros((128, 68), np.float32)
    for col, name in ((0, "c1"), (16, "c2"), (32, "c3"), (48, "c4")):
        b = (inputs[f"bih_{name}"].astype(np.float32)
             + inputs[f"bhh_{name}"].astype(np.float32))
        bias[:, col:col + 16] = b.reshape(16, 128).T
    bias[:, 64:68] = inputs["b_out"].astype(np.float32).reshape(4, 128).T

    state = np.asarray(inputs["state"], np.float32)
    goal = np.asarray(inputs["goal"], np.float32)
    u = np.asarray(inputs["u"], np.float32)

    in_maps = []
    for c in range(N_CORES):
        sl = slice(c * BL, (c + 1) * BL)
        xc = np.concatenate([state[sl], goal[sl]], axis=-1)       # [BL, T, IN_DIM]
        # [T, 2, 128, KX/2, BL]: xp[t, s, p, kk, b] = xc[b, t, (s*KX/2+kk)*128+p]
        xp = np.ascontiguousarray(
            xc.transpose(1, 2, 0).reshape(T, KX, 128, BL).transpose(0, 2, 1, 3)
            .reshape(T, 128, KX, BL)
        ).astype(E4M3).transpose(0, 2, 1, 3).reshape(T, 2, KX // 2, 128, BL) \
            .transpose(0, 1, 3, 2, 4)
        xp = np.ascontiguousarray(xp)
        # u: [BL, ACT] -> [128, NR*BL]: up[p, r*BL+b] = u[b, r*128+p]
        up = np.ascontiguousarray(
            u[sl].T.reshape(NR, 128, BL).transpose(1, 0, 2).reshape(128, NR * BL),
            dtype=np.float32,
        )
        in_maps.append({
            "x": xp, "w1": w1, "wh1": wh1, "w2": w2, "w3": w3, "w4": w4,
            "wo": wo, "bias": bias, "u": up,
        })
    return in_maps


def assemble_output(results: list[dict]) -> np.ndarray:
    out = np.empty((B, ACT_DIM), np.float32)
    for c in range(N_CORES):
        a = results[c]["out"].reshape(ACT_DIM, BL)    # [acts, batch]
        out[c * BL:(c + 1) * BL] = a.T
    return out


_NC_CACHE = None


def kernel(**inputs) -> np.ndarray:
    global _NC_CACHE
    import os

    from concourse.bass_utils import run_bass_kernel_spmd

    # profiling shims aren't installed here; never let an inherited
    # BASS_TRACE flip run_bass_kernel_spmd into the trace path
    os.environ["BASS_NEVER_TRACE"] = "1"

    inputs = {k: np.asarray(v) for k, v in inputs.items()}
    if _NC_CACHE is None:
        _NC_CACHE = build_kernel()
    in_maps = prep_core_inputs(inputs)
    res = run_bass_kernel_spmd(_NC_CACHE, in_maps, core_ids=list(range(N_CORES)))
    return assemble_output(res.results)


if __name__ == "__main__":
    import reference

    inputs = reference.setup_inputs()
    inputs = {k: np.asarray(v) for k, v in inputs.items()}
    got = kernel(**inputs)
    want = np.asarray(reference.reference(**inputs))
    flips = (got != want).sum()
    print("flips:", int(flips), "rel_err:",
          np.linalg.norm(got - want) / max(np.linalg.norm(want), 1e-30))


# revision 21
# speedup vs baseline: 1.7500x; 1.6045x over previous
"""Trainium2 Bass kernel for nn_Actor (MTRNN actor: 4-step LSTM stack + Bernoulli head).

Data-parallel over 8 NeuronCores: batch 4096 -> 512 rows/core, weights replicated.
Everything on-chip lives in [feature, batch] (transposed) layout; all transposes and
weight tiling happen on the host so the NEFF contains only matmuls + elementwise.

Dead-code elimination: in the reference, h2=lstm2(h1) is taken at t==1, h3 at t==2
from h2, h4 at t==3 from h3 — so LSTM1's t=2/t=3 cells (and x[:,2:], and c1 after
t=1) never reach the output. Verified bit-exact against the full forward on host.
Only t=0 (i/g/o gates) and t=1 (full cell) of LSTM1 are computed.

Precision: LSTM1 (x-matmul + recurrent) in fp8 e4m3 operands with DoubleRow perf
mode (2 fp8 MACs per PE cell per cycle) and fp32 PSUM accumulation; weights are
host-scaled by 64 to stay in e4m3 normal range (undone by the activation's scale).
LSTM2-4 stay bf16 and the output head fp32 — fp8 there costs action-bit flips
(validated host-side: this split flips ~2-5 of 2.1M action bits vs fp32, inside
the rel_err 2e-2 gate).
"""
import sys
from contextlib import ExitStack

import numpy as np

sys.path.insert(0, "/opt/trn_rl_repo")

import ml_dtypes

import concourse.bass as bass
import concourse.tile as tile
from concourse import bacc, mybir
from concourse.vector_clock import ScopedClock

BF16 = ml_dtypes.bfloat16
E4M3 = ml_dtypes.float8_e4m3

H = 512
T = 4
TL = 2                     # live LSTM1 steps (t=2,3 are dead code)
IN_DIM = 3072
ACT_DIM = 512
B = 4096
N_CORES = 8
BL = B // N_CORES          # 512 batch rows per core
KX = IN_DIM // 128         # 24 input-feature chunks
KP = KX // 2               # 12 DoubleRow k-pairs
KH = KP // 2               # 6 k-pairs per x half-tile
NM = 16                    # gate chunks (4H/128)
NR = 4                     # H row chunks
WSCALE = 64.0              # host-side weight scale into e4m3 normal range

F32 = mybir.dt.float32
BF = mybir.dt.bfloat16
FP8 = mybir.dt.float8e4
Act = mybir.ActivationFunctionType
Alu = mybir.AluOpType
DR = mybir.MatmulPerfMode.DoubleRow

# w1 gate-chunk pairs per DMA tile, in t0 usage order (i/g/o chunks first,
# f chunks last — they are first needed at t1)
W1PAIRS = ((0, 8), (12, 1), (9, 13), (2, 10), (14, 3), (11, 15), (4, 5), (6, 7))


# ---------------------------------------------------------------------------
# TileContext drain patch: this walrus caps sync-waits per instruction, while
# the stock Tile exit puts one wait per live semaphore on a single Drain.
# Redistribute: one nop per wait, then a wait-free drain.
# ---------------------------------------------------------------------------
def _split_drain_and_barrier(self, tick_clock, wait_clock):
    nc = self.nc
    collector = nc.sync.nop(nofuse=True)
    wait_clock.add_sem_waits(collector.ins, ScopedClock({None: tick_clock.global_clock}))
    si = collector.ins.sync_info
    waits = list(si.on_wait) if si is not None else []
    if len(waits) > 1:
        collector.ins.sync_info = None
        id2sem = {h.num: h for h in self.sems.allocated().values()}
        for w in waits:
            nc.sync.nop(nofuse=True).wait_op(id2sem[w.id], w.wait_value, "sem-ge")
    nc.sync.drain()
    nc.all_engine_barrier()
    assert self.sems is not None
    popped = nc._tile_sem_poison_stack.pop()
    assert popped is self._sem_poison
    nc.clear_and_free_semaphores(list(self.sems.allocated().values()))
    nc.all_engine_barrier()


tile.TileContext._drain_and_barrier = _split_drain_and_barrier


def _chunk(role: str, r: int) -> int:
    """Gate chunk index for role in torch LSTM order [i, f, g, o]."""
    return {"i": 0, "f": 1, "g": 2, "o": 3}[role] * NR + r


def build_kernel() -> bass.Bass:
    nc = bacc.Bacc()

    x_ext = nc.declare_dram_parameter("x", [TL, 2, 128, 2 * KH, BL], FP8, isOutput=False)
    w1_ext = nc.declare_dram_parameter("w1", [NM // 2, 128, KX, 2, 128], FP8,
                                       isOutput=False)
    wh1_ext = nc.declare_dram_parameter("wh1", [128, NM * 4, 128], FP8, isOutput=False)
    w2_ext = nc.declare_dram_parameter("w2", [128, NM * 512], BF, isOutput=False)
    w3_ext = nc.declare_dram_parameter("w3", [128, NM * 512], BF, isOutput=False)
    w4_ext = nc.declare_dram_parameter("w4", [128, NM * 512], BF, isOutput=False)
    wo_ext = nc.declare_dram_parameter("wo", [128, NR * 512], F32, isOutput=False)
    bias_ext = nc.declare_dram_parameter("bias", [128, 68], F32, isOutput=False)
    u_ext = nc.declare_dram_parameter("u", [128, NR * BL], F32, isOutput=False)
    out_ext = nc.declare_dram_parameter("out", [NR, 128, BL], F32, isOutput=True)

    with ExitStack() as ctx:
        tc = ctx.enter_context(tile.TileContext(nc))
        pers = ctx.enter_context(tc.tile_pool(name="pers", bufs=1))
        gate = ctx.enter_context(tc.tile_pool(name="gate", bufs=12))
        ps = ctx.enter_context(tc.tile_pool(name="ps", bufs=8, space="PSUM"))

        def load_x_step(t):
            tiles = []
            for s in range(2):
                xt = pers.tile([128, 2 * KH, BL], FP8, name=f"x_t{t}_{s}",
                               tag=f"xh{s}", bufs=2)
                # both steps' x go down the sync HWDGE ring (starts earliest)
                nc.sync.dma_start(xt[:], x_ext[t][s])
                tiles.append(xt)
            return tiles

        x_step = load_x_step(0)
        bias_sb = pers.tile([128, 68], F32, name="bias", tag="bias")
        nc.gpsimd.dma_start(bias_sb[:], bias_ext[:])
        # single w1 tile (one semaphore), filled by 8 double-chunk DMAs
        # (6KB/partition descriptors — the HW rings crawl at ~85GB/s on 3KB
        # lines but >~200GB/s on larger ones), in t0 usage order: 4 i/g/o
        # pair-tiles stream on the scalar ring, the rest go to gpsimd
        w1_sb = pers.tile([128, NM // 2, KX, 2, 128], FP8, name="w1", tag="w1")
        w1_slot = {}            # gate-chunk m -> (pair idx, local slot)
        for g, pair in enumerate(W1PAIRS):
            eng = nc.scalar if g < 4 else nc.gpsimd
            eng.dma_start(w1_sb[:, g], w1_ext[g])
            for ml, m in enumerate(pair):
                w1_slot[m] = (g, ml)

        warm = pers.tile([128, 1], F32, name="warm", tag="warm")
        nc.scalar.activation(warm[:], bias_sb[:, 0:1], Act.Sigmoid)
        nc.scalar.activation(warm[:], bias_sb[:, 0:1], Act.Tanh)
        dmy = pers.tile([128, BL], BF, name="dmy", tag="dmy")
        nc.vector.memset(dmy[:], 0.0)
        dmy_p = ps.tile([128, BL], F32, name="dmy_p", tag="psum")
        for _ in range(12):
            nc.tensor.matmul(dmy_p[:], dmy[:, 0:128], dmy[:], start=True, stop=True)
        nc.scalar.activation(warm[:], dmy_p[:, 0:1], Act.Relu)
        wh1_sb = pers.tile([128, NM * 4, 128], FP8, name="wh1", tag="wh1")
        nc.gpsimd.dma_start(wh1_sb[:], wh1_ext[:])

        # persistent state
        c1 = pers.tile([128, NR, BL], F32, name="c1", tag="c1")
        h1 = pers.tile([128, NR, BL], BF, name="h1", tag="h1")     # t1 h (lblock2 in)
        h1p = pers.tile([128, NR, BL], FP8, name="h1p", tag="h1p")  # t0 h (recurrent in)
        h2 = pers.tile([128, NR, BL], BF, name="h2", tag="h2")
        h3 = pers.tile([128, NR, BL], BF, name="h3", tag="h3")
        h4 = pers.tile([128, NR, BL], mybir.dt.float32r, name="h4", tag="h4")
        wl_tiles = {}
        INV = 1.0 / WSCALE

        def bias_ap(col):
            return bias_sb[:, col:col + 1]

        def emit_lblock(idx, src_h):
            wl = wl_tiles[("w2", "w3", "w4")[idx]]
            dst = (h2, h3, h4)[idx]
            bias_off = 16 * (idx + 1)
            for half in ((0, 1), (2, 3)):
                psums = {}
                # k-split: the k=0,1 matmuls only need the first half of
                # src_h, so they issue while its second half is still in the
                # producer's gate chain
                for ks in ((0, 1), (2, 3)):
                    for r in half:
                        for role in ("i", "g", "o"):   # f-gate unused (c_prev=0)
                            m = _chunk(role, r)
                            if (role, r) not in psums:
                                psums[(role, r)] = ps.tile([128, BL], F32,
                                                           name="psum", tag="psum")
                            p = psums[(role, r)]
                            for k in ks:
                                nc.tensor.matmul(
                                    p[:],
                                    wl[:, m * 512 + k * 128:m * 512 + (k + 1) * 128],
                                    src_h[:, k, :],
                                    start=(k == 0),
                                    stop=(k == NR - 1),
                                )
                for r in half:
                    si = gate.tile([128, BL], F32, name="si", tag="gt")
                    nc.scalar.activation(si[:], psums[("i", r)][:], Act.Sigmoid,
                                         bias=bias_ap(bias_off + _chunk("i", r)))
                    tg = gate.tile([128, BL], F32, name="tg", tag="gt")
                    nc.scalar.activation(tg[:], psums[("g", r)][:], Act.Tanh,
                                         bias=bias_ap(bias_off + _chunk("g", r)))
                    so = gate.tile([128, BL], F32, name="so", tag="gt")
                    nc.scalar.activation(so[:], psums[("o", r)][:], Act.Sigmoid,
                                         bias=bias_ap(bias_off + _chunk("o", r)))
                    cn = gate.tile([128, BL], F32, name="ig", tag="gt")
                    nc.vector.tensor_tensor(cn[:], si[:], tg[:], Alu.mult)
                    tc_ = gate.tile([128, BL], F32, name="tc", tag="gt")
                    nc.scalar.activation(tc_[:], cn[:], Act.Tanh)
                    nc.vector.scalar_tensor_tensor(dst[:, r, :], tc_[:], 0.0, so[:],
                                                   Alu.max, Alu.mult)

        # --- LSTM1: 2 live steps (fp8 DoubleRow matmuls) ----------------------
        for t in range(TL):
            roles = ("i", "g", "o") if t == 0 else ("i", "f", "g", "o")
            x_cur = x_step
            if t == 0:
                x_step = load_x_step(1)
                # prefetch L-block weights + head tensors on gpsimd
                for name, ext in (("w2", w2_ext), ("w3", w3_ext), ("w4", w4_ext)):
                    wlt = pers.tile([128, NM * 512], BF, name=name, tag=name)
                    nc.gpsimd.dma_start(wlt[:], ext[:])
                    wl_tiles[name] = wlt
                wo_sb = pers.tile([128, NR * 512], mybir.dt.float32r, name="wo", tag="wo")
                nc.gpsimd.dma_start(wo_sb[:], wo_ext[:])
                u_sb = pers.tile([128, NR * BL], F32, name="u", tag="u")
                nc.gpsimd.dma_start(u_sb[:], u_ext[:])

            for half in ((0, 1), (2, 3)):
                psums = {}

                def xs(kp):
                    # rhs pair kp (0..KP-1): [128, 2, BL] slice of an x half-tile
                    s, kk = kp // KH, kp % KH
                    return x_cur[s][:, 2 * kk:2 * kk + 2, :]

                def emit_group(r, role, kps):
                    m = _chunk(role, r)
                    if (role, r) not in psums:
                        psums[(role, r)] = ps.tile([128, BL], F32,
                                                   name="psum", tag="psum")
                    p = psums[(role, r)]
                    g, ml = w1_slot[m]
                    for kp in kps:
                        nc.tensor.matmul(
                            p[:],
                            w1_sb[:, g, 2 * kp:2 * kp + 2, ml, :],
                            xs(kp),
                            start=(kp == 0),
                            stop=(t == 0 and kp == KP - 1),
                            perf_mode=DR,
                        )

                grps = [(r, role) for r in half for role in roles]
                if t == 0 and half == (0, 1):
                    # staged warm-start: the first pair-tiles and x halves
                    # arrive while the PE chews the earlier stages
                    for r, role in grps[:2]:            # w1 pair-tile 0 only
                        emit_group(r, role, range(KH))
                    for r, role in grps[2:4]:           # + pair-tile 1
                        emit_group(r, role, range(KH))
                    for r, role in grps[:4]:            # + x second half
                        emit_group(r, role, range(KH, KP))
                    for r, role in grps[4:]:
                        emit_group(r, role, range(KP))
                else:
                    for r, role in grps:
                        emit_group(r, role, range(KP))
                # recurrent matmuls accumulate into the same PSUM groups
                if t > 0:
                    for r in half:
                        for role in roles:
                            m = _chunk(role, r)
                            p = psums[(role, r)]
                            for kp in range(2):
                                nc.tensor.matmul(
                                    p[:],
                                    wh1_sb[:, m * 4 + 2 * kp:m * 4 + 2 * kp + 2, :],
                                    h1p[:, 2 * kp:2 * kp + 2, :],
                                    start=False,
                                    stop=(kp == 1),
                                    perf_mode=DR,
                                )
                # gate nonlinearities + state update per row
                for r in half:
                    si = gate.tile([128, BL], F32, name="si", tag="gt")
                    nc.scalar.activation(si[:], psums[("i", r)][:], Act.Sigmoid,
                                         bias=bias_ap(_chunk("i", r)), scale=INV)
                    tg = gate.tile([128, BL], F32, name="tg", tag="gt")
                    nc.scalar.activation(tg[:], psums[("g", r)][:], Act.Tanh,
                                         bias=bias_ap(_chunk("g", r)), scale=INV)
                    so = gate.tile([128, BL], F32, name="so", tag="gt")
                    nc.scalar.activation(so[:], psums[("o", r)][:], Act.Sigmoid,
                                         bias=bias_ap(_chunk("o", r)), scale=INV)
                    ig = gate.tile([128, BL], F32, name="ig", tag="gt")
                    nc.vector.tensor_tensor(ig[:], si[:], tg[:], Alu.mult)
                    if t == 0:
                        cn = ig
                    else:
                        sf = gate.tile([128, BL], F32, name="sf", tag="gt")
                        nc.scalar.activation(sf[:], psums[("f", r)][:], Act.Sigmoid,
                                             bias=bias_ap(_chunk("f", r)), scale=INV)
                        fc = gate.tile([128, BL], F32, name="fc", tag="gt")
                        nc.vector.tensor_tensor(fc[:], sf[:], c1[:, r, :], Alu.mult)
                        cn = gate.tile([128, BL], F32, name="cn", tag="gt")
                        nc.vector.tensor_tensor(cn[:], fc[:], ig[:], Alu.add)
                    # h = so * relu(tanh(cn))  (== relu(so*tanh(relu(cn))):
                    # so > 0, tanh monotone); t0's h goes straight to fp8 for
                    # the t1 recurrence, t1's h to bf16 for lblock2.
                    # c1 is only live into t1's f-gate.
                    tc_ = gate.tile([128, BL], F32, name="tc", tag="gt")
                    nc.scalar.activation(tc_[:], cn[:], Act.Tanh)
                    if t == 0:
                        nc.vector.tensor_scalar_max(c1[:, r, :], cn[:], 0.0)
                        nc.vector.scalar_tensor_tensor(h1p[:, r, :], tc_[:], 0.0,
                                                       so[:], Alu.max, Alu.mult)
                    else:
                        nc.vector.scalar_tensor_tensor(h1[:, r, :], tc_[:], 0.0,
                                                       so[:], Alu.max, Alu.mult)

        emit_lblock(0, h1)
        emit_lblock(1, h2)
        emit_lblock(2, h3)

        # --- output head: f32r matmul + relu + Bernoulli threshold ------------
        # k-split like the lblocks: k=0,1 issue as soon as h4's first half is
        # out of lblock4's gate chain
        hps = []
        for ks in ((0, 1), (2, 3)):
            for r in range(NR):
                if ks == (0, 1):
                    hps.append(ps.tile([128, BL], F32, name="psum", tag="psum"))
                p = hps[r]
                for k in ks:
                    nc.tensor.matmul(
                        p[:],
                        wo_sb[:, r * 512 + k * 128:r * 512 + (k + 1) * 128],
                        h4[:, k, :],
                        start=(k == 0),
                        stop=(k == NR - 1),
                    )
        for r in range(NR):
            probs = gate.tile([128, BL], F32, name="probs", tag="gt")
            nc.scalar.activation(probs[:], hps[r][:], Act.Relu, bias=bias_ap(64 + r))
            act = gate.tile([128, BL], F32, name="act", tag="gt")
            nc.vector.tensor_tensor(act[:], probs[:], u_sb[:, r * BL:(r + 1) * BL], Alu.is_gt)
            nc.sync.dma_start(out_ext[r], act[:])

    nc.finalize()
    return nc


# ---------------------------------------------------------------------------
# Host-side input prep / output assembly
# ---------------------------------------------------------------------------
def _tile_weight(wT: np.ndarray, dtype, scale=1.0) -> np.ndarray:
    """[K, M] (transposed weight) -> [128, M*K/128] where
    arr[p, m*K + k*128 + j] = wT[k*128+p, m*128+j]."""
    K, M = wT.shape
    kc, mc = K // 128, M // 128
    return np.ascontiguousarray(
        (wT * scale).reshape(kc, 128, mc, 128).transpose(1, 2, 0, 3).reshape(128, M * kc)
    ).astype(dtype)


def _tile_weight_w1(wT: np.ndarray, dtype, scale=1.0) -> np.ndarray:
    """[K, M] -> [M/128, 128, K]: arr[m, p, k*128+j] = wT[k*128+p, m*128+j]."""
    K, M = wT.shape
    kc, mc = K // 128, M // 128
    return np.ascontiguousarray(
        (wT * scale).reshape(kc, 128, mc, 128).transpose(2, 1, 0, 3).reshape(mc, 128, K)
    ).astype(dtype)


def prep_core_inputs(inputs: dict) -> list[dict]:
    """Full inputs -> per-core in_maps with host-side transpose/tiling."""
    w1_rs = _tile_weight_w1(np.ascontiguousarray(inputs["Wih_c1"].T), E4M3,
                            scale=WSCALE).reshape(NM, 128, KX, 128)
    # pack gate-chunk pairs per W1PAIRS: [8, 128, KX, 2, 128]
    w1 = np.ascontiguousarray(w1_rs[np.array(W1PAIRS)].transpose(0, 2, 3, 1, 4))
    wh1 = _tile_weight(np.ascontiguousarray(inputs["Whh_c1"].T), E4M3,
                       scale=WSCALE).reshape(128, NM * 4, 128)
    w2 = _tile_weight(np.ascontiguousarray(inputs["Wih_c2"].T), BF16)
    w3 = _tile_weight(np.ascontiguousarray(inputs["Wih_c3"].T), BF16)
    w4 = _tile_weight(np.ascontiguousarray(inputs["Wih_c4"].T), BF16)
    wo = _tile_weight(np.ascontiguousarray(inputs["W_out"].T.astype(np.float32)), np.float32)

    bias = np.zeros((128, 68), np.float32)
    for col, name in ((0, "c1"), (16, "c2"), (32, "c3"), (48, "c4")):
        b = (inputs[f"bih_{name}"].astype(np.float32)
             + inputs[f"bhh_{name}"].astype(np.float32))
        bias[:, col:col + 16] = b.reshape(16, 128).T
    bias[:, 64:68] = inputs["b_out"].astype(np.float32).reshape(4, 128).T

    # only t=0,1 of the input sequence are live
    state = np.asarray(inputs["state"], np.float32)[:, :TL]
    goal = np.asarray(inputs["goal"], np.float32)[:, :TL]
    u = np.asarray(inputs["u"], np.float32)

    in_maps = []
    for c in range(N_CORES):
        sl = slice(c * BL, (c + 1) * BL)
        xc = np.concatenate([state[sl], goal[sl]], axis=-1)       # [BL, TL, IN_DIM]
        # [TL, 2, 128, KX/2, BL]: xp[t, s, p, kk, b] = xc[b, t, (s*KX/2+kk)*128+p]
        xp = np.ascontiguousarray(
            xc.transpose(1, 2, 0).reshape(TL, KX, 128, BL).transpose(0, 2, 1, 3)
            .reshape(TL, 128, KX, BL)
        ).astype(E4M3).transpose(0, 2, 1, 3).reshape(TL, 2, KX // 2, 128, BL) \
            .transpose(0, 1, 3, 2, 4)
        xp = np.ascontiguousarray(xp)
        # u: [BL, ACT] -> [128, NR*BL]: up[p, r*BL+b] = u[b, r*128+p]
        up = np.ascontiguousarray(
            u[sl].T.reshape(NR, 128, BL).transpose(1, 0, 2).reshape(128, NR * BL),
            dtype=np.float32,
        )
        in_maps.append({
            "x": xp, "w1": w1, "wh1": wh1, "w2": w2, "w3": w3, "w4": w4,
            "wo": wo, "bias": bias, "u": up,
        })
    return in_maps


def assemble_output(results: list[dict]) -> np.ndarray:
    out = np.empty((B, ACT_DIM), np.float32)
    for c in range(N_CORES):
        a = results[c]["out"].reshape(ACT_DIM, BL)    # [acts, batch]
        out[c * BL:(c + 1) * BL] = a.T
    return out


_NC_CACHE = None


def kernel(**inputs) -> np.ndarray:
    global _NC_CACHE
    import os

    from concourse.bass_utils import run_bass_kernel_spmd

    # profiling shims aren't installed here; never let an inherited
    # BASS_TRACE flip run_bass_kernel_spmd into the trace path
    os.environ["BASS_NEVER_TRACE"] = "1"

    inputs = {k: np.asarray(v) for k, v in inputs.items()}
    if _NC_CACHE is None:
        _NC_CACHE = build_kernel()
    in_maps = prep_core_inputs(inputs)
    res = run_bass_kernel_spmd(_NC_CACHE, in_maps, core_ids=list(range(N_CORES)))
    return assemble_output(res.results)


if __name__ == "__main__":
    import reference

    inputs = reference.setup_inputs()
    inputs = {k: np.asarray(v) for k, v in inputs.items()}
    got = kernel(**inputs)
    want = np.asarray(reference.reference(**inputs))
    flips = (got != want).sum()
    print("flips:", int(flips), "rel_err:",
          np.linalg.norm(got - want) / max(np.linalg.norm(want), 1e-30))
